# revision 1
# baseline (speedup 1.0000x reference)
"""Trainium2 Bass kernel for LILayer: y = scan(decay, (1-decay)*(x @ W.T + b)).

Strategy
--------
Data-parallel over batch B=8 across the 8 NeuronCores (one batch row each).
Per core:
  - GEMM  C[o, t] = sum_i W'[o, i] * x[t, i]  with W' = (1-decay)[:, None] * W
    (the (1-decay) factor and the transposes are folded in on the host, so the
    device sees xT = x[b].T  [IN, T]  and wS = W'.T  [IN, OUT], both with the
    contraction dim IN on SBUF partitions and fully-contiguous DMA).
  - Recurrence v_t = decay * v_{t-1} + C_t  via the native VectorE
    tensor_tensor_scan (op0=mult, op1=add) along the free (time) dim,
    chained across time chunks through the previous chunk's last column.
  - Output per core: out[o, t] (f32).  Host transposes back to [T, OUT].
Bias enters the recurrence linearly, so its exact contribution
b * (1 - decay**(t+1)) is added in closed form on the host (bias is zero in
this problem's input spec, making that a numeric no-op).
"""

import numpy as np

_B, _T, _IN, _OUT = 8, 2048, 1024, 512
_P = 128               # SBUF partitions
_KI = _IN // _P        # 8 contraction tiles
_G = _OUT // _P        # 4 output-feature groups
_TCH = 512             # time-chunk (one PSUM bank at fp32)
_NCH = _T // _TCH      # 4 chunks
_NCORES = 8
_USE_BF16 = True       # matmul operand dtype (scan/accum stay fp32)

_compiled_nc = None


def _build():
    import concourse.bacc as bacc
    import concourse.mybir as mybir
    import concourse.tile as tile

    f32 = mybir.dt.float32
    dt_in = mybir.dt.bfloat16 if _USE_BF16 else f32

    nc = bacc.Bacc("TRN2", target_bir_lowering=False, debug=False)
    xT = nc.declare_dram_parameter("xT", [_IN, _T], dt_in, isOutput=False)
    wS = nc.declare_dram_parameter("wS", [_IN, _OUT], dt_in, isOutput=False)
    dec = nc.declare_dram_parameter("dec", [_OUT], f32, isOutput=False)
    out = nc.declare_dram_parameter("out", [_OUT, _T], f32, isOutput=True)

    with tile.TileContext(nc) as tc:
        with (
            tc.tile_pool(name="const", bufs=1) as cp,
            tc.tile_pool(name="xp", bufs=3) as xp,
            tc.tile_pool(name="vp", bufs=10) as vp,
            tc.tile_pool(name="pp", bufs=8, space="PSUM") as pp,
        ):
            xT_r = xT[:, :].rearrange("(ki p) t -> p ki t", p=_P)
            wS_r = wS[:, :].rearrange("(ki p) o -> p ki o", p=_P)

            dec_sb = cp.tile([_P, _G], f32, tag="dec")
            nc.sync.dma_start(dec_sb[:], dec[:].rearrange("(g p) -> p g", p=_P))

            w_sb = cp.tile([_P, _KI, _OUT], dt_in, tag="w")
            for ki in range(_KI):
                nc.sync.dma_start(w_sb[:, ki, :], wS_r[:, ki, :])

            # decay broadcast along the time axis for the scan's data0
            ones = cp.tile([_P, _TCH], f32, tag="ones")
            nc.vector.memset(ones[:], 1.0)
            dbc = cp.tile([_P, _G, _TCH], f32, tag="dbc")
            for g in range(_G):
                nc.vector.tensor_scalar_mul(dbc[:, g, :], ones[:], dec_sb[:, g : g + 1])

            prev = [None] * _G
            for c in range(_NCH):
                x_c = xp.tile([_P, _KI, _TCH], dt_in, tag="x")
                half = _KI // 2
                for h in range(2):
                    nc.sync.dma_start(
                        x_c[:, h * half : (h + 1) * half, :],
                        xT_r[:, h * half : (h + 1) * half, c * _TCH : (c + 1) * _TCH],
                    )
                for g in range(_G):
                    ps = pp.tile([_P, _TCH], f32, tag="ps")
                    for ki in range(_KI):
                        nc.tensor.matmul(
                            ps[:],
                            w_sb[:, ki, g * _P : (g + 1) * _P],
                            x_c[:, ki, :],
                            start=(ki == 0),
                            stop=(ki == _KI - 1),
                        )
                    v = vp.tile([_P, _TCH], f32, tag="v")
                    init = 0.0 if prev[g] is None else prev[g][:, _TCH - 1 : _TCH]
                    nc.vector.tensor_tensor_scan(
                        v[:],
                        dbc[:, g, :],
                        ps[:],
                        init,
                        mybir.AluOpType.mult,
                        mybir.AluOpType.add,
                    )
                    prev[g] = v
                    nc.sync.dma_start(
                        out[g * _P : (g + 1) * _P, c * _TCH : (c + 1) * _TCH], v[:]
                    )

    nc.compile()
    return nc


def _get_nc():
    global _compiled_nc
    if _compiled_nc is None:
        _compiled_nc = _build()
    return _compiled_nc


def kernel(input_tensor, weight, bias, decay):
    from concourse.bass_utils import run_bass_kernel_spmd

    x = np.asarray(input_tensor, dtype=np.float32)
    w = np.asarray(weight, dtype=np.float32)
    b = np.asarray(bias, dtype=np.float32)
    d = np.asarray(decay, dtype=np.float32)

    if _USE_BF16:
        import ml_dtypes

        in_np_dt = ml_dtypes.bfloat16
    else:
        in_np_dt = np.float32

    wS = np.ascontiguousarray(((1.0 - d)[:, None] * w).T.astype(in_np_dt))  # [IN, OUT]
    xT = np.ascontiguousarray(np.transpose(x, (0, 2, 1))).astype(in_np_dt)  # [B, IN, T]

    nc = _get_nc()
    in_maps = [{"xT": xT[i], "wS": wS, "dec": d} for i in range(_B)]
    res = run_bass_kernel_spmd(nc, in_maps, core_ids=list(range(_NCORES))).results

    outputs = np.stack([res[i]["out"].T for i in range(_B)], axis=0)  # [B, T, OUT]
    outputs = np.ascontiguousarray(outputs, dtype=np.float32)

    if np.any(b != 0.0):
        k = np.arange(1, _T + 1, dtype=np.float64)[None, :, None]
        outputs = (
            outputs
            + (b[None, None, :] * (1.0 - d.astype(np.float64)[None, None, :] ** k))
        ).astype(np.float32)

    states = np.concatenate(
        [np.zeros((_B, 1, _OUT), np.float32), outputs], axis=1
    )[None]
    return outputs, states


# revision 2
# speedup vs baseline: 1.0975x; 1.0975x over previous
"""Trainium2 Bass kernel for LILayer: y = scan(decay, (1-decay)*(x @ W.T + b)).

Strategy
--------
Data-parallel over batch B=8 across the 8 NeuronCores (one batch row each).
Per core:
  - GEMM  C[o, t] = sum_i W'[o, i] * x[t, i]  with W' = (1-decay)[:, None] * W
    (the (1-decay) factor and the transposes are folded in on the host, so the
    device sees xT = x[b].T  [IN, T]  and wS = W'.T  [IN, OUT], both with the
    contraction dim IN on SBUF partitions and fully-contiguous DMA).
  - Recurrence v_t = decay * v_{t-1} + C_t  via the native VectorE
    tensor_tensor_scan (op0=mult, op1=add) along the free (time) dim,
    chained across time chunks through the previous chunk's last column.
  - Output per core: out[o, t] (f32).  Host transposes back to [T, OUT].
Bias enters the recurrence linearly, so its exact contribution
b * (1 - decay**(t+1)) is added in closed form on the host (bias is zero in
this problem's input spec, making that a numeric no-op).

Perf notes (from NTFF traces):
  - loads go on the Sync HWDGE ring, stores + weights on the Scalar ring so
    issue cost (~0.6us per dma_start) is split across both sequencers.
  - x is loaded as 16 half-T per-ki tiles so the first matmul can start
    ~2us after the preamble instead of waiting for the whole 4MB.
  - matmul pitch at N=512 is ~216ns warm; PE time ~27.6us is the roofline.
"""

import numpy as np

_B, _T, _IN, _OUT = 8, 2048, 1024, 512
_P = 128               # SBUF partitions
_KI = _IN // _P        # 8 contraction tiles
_G = _OUT // _P        # 4 output-feature groups
_TCH = 512             # time-chunk (one PSUM bank at fp32)
_NCH = _T // _TCH      # 4 chunks
_HALF = _T // 2        # half-T span per x load
_NCORES = 8
_USE_BF16 = True       # matmul operand dtype (scan/accum stay fp32)

_compiled_nc = None


def _build():
    import concourse.bacc as bacc
    import concourse.mybir as mybir
    import concourse.tile as tile

    f32 = mybir.dt.float32
    dt_in = mybir.dt.bfloat16 if _USE_BF16 else f32

    nc = bacc.Bacc("TRN2", target_bir_lowering=False, debug=False)
    xT = nc.declare_dram_parameter("xT", [_IN, _T], dt_in, isOutput=False)
    wS = nc.declare_dram_parameter("wS", [_IN, _OUT], dt_in, isOutput=False)
    dec = nc.declare_dram_parameter("dec", [_OUT], f32, isOutput=False)
    out = nc.declare_dram_parameter("out", [_OUT, _T], f32, isOutput=True)

    with tile.TileContext(nc) as tc:
        with (
            tc.tile_pool(name="const", bufs=1) as cp,
            tc.tile_pool(name="xp", bufs=16) as xp,
            tc.tile_pool(name="vp", bufs=10) as vp,
            tc.tile_pool(name="pp", bufs=8, space="PSUM") as pp,
        ):
            xT_r = xT[:, :].rearrange("(ki p) t -> p ki t", p=_P)
            wS_r = wS[:, :].rearrange("(ki p) o -> p ki o", p=_P)

            # x: 16 half-T tiles, issued ki-major per half so the ki=0..7 set
            # for the first two chunks lands early.  Loads ride the Sync ring.
            x_sb = [[None] * 2 for _ in range(_KI)]
            for h in range(2):
                for ki in range(_KI):
                    t_ = xp.tile([_P, _HALF], dt_in, tag="x")
                    nc.sync.dma_start(
                        t_[:], xT_r[:, ki, h * _HALF : (h + 1) * _HALF]
                    )
                    x_sb[ki][h] = t_

            # weights + dec ride the Scalar (ACT) ring in parallel.
            w_sb = cp.tile([_P, _KI, _OUT], dt_in, tag="w")
            for ki in range(_KI):
                nc.scalar.dma_start(w_sb[:, ki, :], wS_r[:, ki, :])
            dec_sb = cp.tile([_P, _G], f32, tag="dec")
            nc.scalar.dma_start(dec_sb[:], dec[:].rearrange("(g p) -> p g", p=_P))

            # decay broadcast along the time axis for the scan's data0
            ones = cp.tile([_P, _TCH], f32, tag="ones")
            nc.vector.memset(ones[:], 1.0)
            dbc = cp.tile([_P, _G, _TCH], f32, tag="dbc")
            for g in range(_G):
                nc.vector.tensor_scalar_mul(dbc[:, g, :], ones[:], dec_sb[:, g : g + 1])

            prev = [None] * _G
            for c in range(_NCH):
                h, r = divmod(c, 2)
                for g in range(_G):
                    ps = pp.tile([_P, _TCH], f32, tag="ps")
                    for ki in range(_KI):
                        nc.tensor.matmul(
                            ps[:],
                            w_sb[:, ki, g * _P : (g + 1) * _P],
                            x_sb[ki][h][:, r * _TCH : (r + 1) * _TCH],
                            start=(ki == 0),
                            stop=(ki == _KI - 1),
                        )
                    v = vp.tile([_P, _TCH], f32, tag="v")
                    init = 0.0 if prev[g] is None else prev[g][:, _TCH - 1 : _TCH]
                    nc.vector.tensor_tensor_scan(
                        v[:],
                        dbc[:, g, :],
                        ps[:],
                        init,
                        mybir.AluOpType.mult,
                        mybir.AluOpType.add,
                    )
                    prev[g] = v
                    nc.scalar.dma_start(
                        out[g * _P : (g + 1) * _P, c * _TCH : (c + 1) * _TCH], v[:]
                    )

    nc.compile()
    return nc


def _get_nc():
    global _compiled_nc
    if _compiled_nc is None:
        _compiled_nc = _build()
    return _compiled_nc


def kernel(input_tensor, weight, bias, decay):
    from concourse.bass_utils import run_bass_kernel_spmd

    x = np.asarray(input_tensor, dtype=np.float32)
    w = np.asarray(weight, dtype=np.float32)
    b = np.asarray(bias, dtype=np.float32)
    d = np.asarray(decay, dtype=np.float32)

    if _USE_BF16:
        import ml_dtypes

        in_np_dt = ml_dtypes.bfloat16
    else:
        in_np_dt = np.float32

    wS = np.ascontiguousarray(((1.0 - d)[:, None] * w).T.astype(in_np_dt))  # [IN, OUT]
    xT = np.ascontiguousarray(np.transpose(x, (0, 2, 1))).astype(in_np_dt)  # [B, IN, T]

    nc = _get_nc()
    in_maps = [{"xT": xT[i], "wS": wS, "dec": d} for i in range(_B)]
    res = run_bass_kernel_spmd(nc, in_maps, core_ids=list(range(_NCORES))).results

    outputs = np.stack([res[i]["out"].T for i in range(_B)], axis=0)  # [B, T, OUT]
    outputs = np.ascontiguousarray(outputs, dtype=np.float32)

    if np.any(b != 0.0):
        k = np.arange(1, _T + 1, dtype=np.float64)[None, :, None]
        outputs = (
            outputs
            + (b[None, None, :] * (1.0 - d.astype(np.float64)[None, None, :] ** k))
        ).astype(np.float32)

    states = np.concatenate(
        [np.zeros((_B, 1, _OUT), np.float32), outputs], axis=1
    )[None]
    return outputs, states
